# revision 19
# baseline (speedup 1.0000x reference)
"""Multi-head attention (B=4, S=1024, H=1024, 16 heads) on 8 TRN2 NeuronCores.

Sharding: core c = (batch b = c//2, head-group g = c%2). Each core computes
attention for its batch over 8 of the 16 heads (a 512-wide column slice of
the QKV projections) plus the matching row-slice of the output projection.
The two partial output projections per batch are summed on the host
(row-parallel tensor-parallel unshard), where the bias bo is also added.

On-core dataflow (matmuls in f32r except the attention-weight matmul in
bf16; psum accumulation is fp32 throughout):
  QT[hd,s] = Wq_g^T x^T (+bq)   KT likewise (+bk)    V[t,hd] = x Wv_g (+bv)
  logitsT[t,s] per head: contraction over d=64; two heads packed in the PE
                         via tile_position row groups
  expT = exp(logitsT/8 + mask*NEG_INF)   (mask enters as per-partition bias)
  AVT[d,s] & colsum = [V_h | 1]^T @ expT (ones column makes psum row 64 the
                                          softmax denominator)
  attnT = AVT * (1/colsum broadcast)     (broadcast via k=1 ones matmul)
  out[s,n] += attnT-chunk^T @ Wo_g       (partial; host sums core pairs)
"""
import sys

sys.path.insert(0, "/opt/trn_rl_repo")

import ml_dtypes
import numpy as np

import concourse.bass as bass
import concourse.mybir as mybir
import concourse.tile as tile
from concourse import bacc
from concourse.bass_utils import run_bass_kernel_spmd

F32 = mybir.dt.float32
F32R = mybir.dt.float32r
BF16 = mybir.dt.bfloat16
EXPTYPE = BF16

B, S, H = 4, 1024, 1024
NH, HD = 16, 64
HPG = 8            # heads per group (per core)
GW = HPG * HD      # 512: group width
NEG_INF = -2.0 ** 32
NCORES = 8
HC = H // 128      # 8 contraction chunks over hidden
TC = S // 128      # 8 chunks over key positions t
SB = S // 512      # 2 halves of the s (query) axis

Exp = mybir.ActivationFunctionType.Exp


def _build(nrep=1):
    nc = bacc.Bacc("TRN2", target_bir_lowering=False, debug=False)

    xT = nc.dram_tensor("xT", [H, S], F32R, kind="ExternalInput")
    wq = nc.dram_tensor("wq", [H, GW], F32R, kind="ExternalInput")
    wk = nc.dram_tensor("wk", [H, GW], F32R, kind="ExternalInput")
    wv = nc.dram_tensor("wv", [H, GW], F32R, kind="ExternalInput")
    wo = nc.dram_tensor("wo", [GW, S], F32R, kind="ExternalInput")
    mask1 = nc.dram_tensor("mask1", [S], F32, kind="ExternalInput")
    bq1 = nc.dram_tensor("bq1", [GW], F32, kind="ExternalInput")
    bk1 = nc.dram_tensor("bk1", [GW], F32, kind="ExternalInput")
    bv1 = nc.dram_tensor("bv1", [1, GW], F32, kind="ExternalInput")
    ones = nc.dram_tensor("ones", [128, TC, HPG, 1], EXPTYPE, kind="ExternalInput")
    onecol = nc.dram_tensor("onecol", [1, HD], F32R, kind="ExternalInput")
    out = nc.dram_tensor("out", [S, H], F32, kind="ExternalOutput")

    with tile.TileContext(nc, pool_alloc_mode="stack") as tc:
      for _rep in range(nrep):
          # Pool releases must be LIFO, so the three big input pools
          # (xT/wqk/wv) are created LAST: they release mid-kernel (stack
          # rewinds) and p_wo then reuses their space.
          misc_cm = tc.tile_pool(name="misc", bufs=1); misc = misc_cm.__enter__()
          qkt_cm = tc.tile_pool(name="p_qkt", bufs=1); p_qkt = qkt_cm.__enter__()
          v_cm = tc.tile_pool(name="p_v", bufs=1); p_v = v_cm.__enter__()
          exp_cm = tc.tile_pool(name="p_exp", bufs=3); p_exp = exp_cm.__enter__()
          attn_cm = tc.tile_pool(name="p_attn", bufs=1)
          p_attn = attn_cm.__enter__()
          nrm_cm = tc.tile_pool(name="p_nrm", bufs=2); p_nrm = nrm_cm.__enter__()
          o_cm = tc.tile_pool(name="p_o", bufs=3); p_o = o_cm.__enter__()
          xT_cm = tc.tile_pool(name="p_xT", bufs=1); p_xT = xT_cm.__enter__()
          wqk_cm = tc.tile_pool(name="p_wqk", bufs=1); p_wqk = wqk_cm.__enter__()
          wv_cm = tc.tile_pool(name="p_wv", bufs=1); p_wv = wv_cm.__enter__()
          late = {"p_nrm": p_nrm}
          qkvps_cm = tc.tile_pool(name="ps_qkv", bufs=2, space="PSUM")
          ps_qkv = qkvps_cm.__enter__()
          lgps_cm = tc.tile_pool(name="ps_lg", bufs=2, space="PSUM")
          ps_lg = lgps_cm.__enter__()
          avps_cm = tc.tile_pool(name="ps_av", bufs=2, space="PSUM")
          ps_av = avps_cm.__enter__()

          # ---- tiny const DMAs first (cheap; they gate exp and copies) ----
          maskb = misc.tile([128, TC], F32, tag="maskb")
          mraw = misc.tile([128, TC], F32, tag="mraw")
          nc.sync.dma_start(out=mraw, in_=mask1.ap().rearrange("(c p) -> p c", p=128))
          nc.vector.tensor_scalar_mul(maskb, mraw, NEG_INF)
          bq_sb = misc.tile([128, 4], F32, tag="bq")
          bk_sb = misc.tile([128, 4], F32, tag="bk")
          nc.sync.dma_start(out=bq_sb, in_=bq1.ap().rearrange("(c p) -> p c", p=128))
          nc.sync.dma_start(out=bk_sb, in_=bk1.ap().rearrange("(c p) -> p c", p=128))
          bv_bc = misc.tile([128, GW], F32, tag="bv")
          nc.sync.dma_start(out=bv_bc, in_=bv1[0:1, :].to_broadcast((128, GW)))
          onecol_sb = misc.tile([1, HD], F32R, tag="onecol")
          nc.sync.dma_start(out=onecol_sb, in_=onecol[:, :])

          # ---- big input loads first: first QT matmul needs xT + wq blk0,
          # so those DMAs get top scheduler priority; weights split per block.
          xT_sb = p_xT.tile([128, HC, S], F32R, tag="xT")
          for sh in range(SB):
              for c in range(HC):
                  nc.sync.dma_start(
                      out=xT_sb[:, c, sh * 512:(sh + 1) * 512],
                      in_=xT[c * 128:(c + 1) * 128, sh * 512:(sh + 1) * 512])
          wq_sb = p_wqk.tile([128, HC, GW], F32R, tag="wq")
          wk_sb = p_wqk.tile([128, HC, GW], F32R, tag="wk")
          wv_sb = p_wv.tile([128, HC, GW], F32R, tag="wv")
          wq_r = wq.ap().rearrange("(c p) m -> p c m", p=128)
          wk_r = wk.ap().rearrange("(c p) m -> p c m", p=128)
          for blk in range(4):
              bs = slice(blk * 128, (blk + 1) * 128)
              nc.sync.dma_start(out=wq_sb[:, :, bs], in_=wq_r[:, :, bs])
          for blk in range(4):
              bs = slice(blk * 128, (blk + 1) * 128)
              nc.sync.dma_start(out=wk_sb[:, :, bs], in_=wk_r[:, :, bs])
          nc.sync.dma_start(out=wv_sb, in_=wv.ap().rearrange("(c p) m -> p c m", p=128))


          QT_sb = p_qkt.tile([128, 4, S], F32R, tag="QT")
          KT_sb = p_qkt.tile([128, 4, S], F32R, tag="KT")
          V_sb = p_v.tile([128, TC, HPG, HD + 1], EXPTYPE, tag="V")
          nc.sync.dma_start(out=V_sb[:, :, :, HD:HD + 1], in_=ones.ap())
          late["attnT"] = p_attn.tile([128, 4, S], F32R, tag="attnT", name="attnT")

          def proj_half(dst, blk, sh, w_sb, b_sb):
              """dst[:, blk, sh-half] (+bias) = block of Wg^T x^T."""
              ps = ps_qkv.tile([128, 512], F32, tag="mm512")
              for c in range(HC):
                  nc.tensor.matmul(
                      ps, w_sb[:, c, blk * 128:(blk + 1) * 128],
                      xT_sb[:, c, sh * 512:(sh + 1) * 512],
                      start=(c == 0), stop=(c == HC - 1))
              nc.vector.tensor_scalar_add(
                  dst[:, blk, sh * 512:(sh + 1) * 512], ps, b_sb[:, blk:blk + 1])

          def v_chunk(tcn):
              """V_sb[:, tcn, :, 0:64] (+bv) = rows 128*tcn.. of x Wv_g."""
              ps = ps_qkv.tile([128, 512], F32, tag="mm512")
              for c in range(HC):
                  nc.tensor.matmul(
                      ps, xT_sb[:, c, tcn * 128:(tcn + 1) * 128], wv_sb[:, c, :],
                      start=(c == 0), stop=(c == HC - 1))
              nc.vector.tensor_add(
                  V_sb[:, tcn, :, 0:HD],
                  ps.rearrange("p (h d) -> p h d", h=HPG),
                  bv_bc.rearrange("p (h d) -> p h d", h=HPG))

          def logits_exp(pair, tcn, exp_dsts):
              """Packed pair of d=64 logit matmuls + exp for chunk tcn."""
              for i, (off, tp) in enumerate(((0, (0, 0)), (64, (64, 0)))):
                  lg = ps_lg.tile([128, 1024], F32, tag="lg")
                  for sh in range(SB):
                      nc.tensor.matmul(
                          lg[:, sh * 512:(sh + 1) * 512],
                          KT_sb[off:off + 64, pair, tcn * 128:(tcn + 1) * 128],
                          QT_sb[off:off + 64, pair, sh * 512:(sh + 1) * 512],
                          start=True, stop=True, tile_position=tp)
                  nc.scalar.activation(
                      out=exp_dsts[i][:, tcn, :], in_=lg, func=Exp,
                      bias=maskb[:, tcn:tcn + 1], scale=0.125)

          def av_head_half(h, expT_h, sh):
              """attnT rows for head h, s-half sh = normalized V_h^T @ expT_h."""
              off = (h % 2) * 64
              if True:
                  pav = ps_av.tile([HD + 1, 512], F32, tag="av",
                                    name=f"pav{h}_{sh}")
                  for tcn in range(TC):
                      nc.tensor.matmul(
                          pav, V_sb[:, tcn, h, :],
                          expT_h[:, tcn, sh * 512:(sh + 1) * 512],
                          start=(tcn == 0), stop=(tcn == TC - 1))
                  recip = late["p_nrm"].tile([1, 512], F32R, tag="recip")
                  with nc.allow_low_precision(reason="softmax denom recip to f32r"):
                      nc.vector.reciprocal(recip, pav[HD:HD + 1, :])
                  bps = ps_av.tile([HD, 512], F32, tag="av",
                                    name=f"bps{h}_{sh}")
                  nc.tensor.matmul(bps, onecol_sb, recip, start=True, stop=True)
                  bcast = late["p_nrm"].tile([HD, 512], F32, tag="bcast")
                  nc.vector.tensor_copy(bcast, bps)
                  nc.vector.tensor_mul(
                      late["attnT"][off:off + HD, h // 2, sh * 512:(sh + 1) * 512],
                      pav[0:HD, :], bcast)

          # ---------------- emission ----------------
          for sh in range(SB):
              proj_half(QT_sb, 0, sh, wq_sb, bq_sb)
          for sh in range(SB):
              proj_half(KT_sb, 0, sh, wk_sb, bk_sb)

          expT = {}
          for pair in range(4):
              ha, hb = 2 * pair, 2 * pair + 1
              expT[ha] = p_exp.tile([128, TC, S], EXPTYPE, tag="expT", name=f"expT{ha}")
              expT[hb] = p_exp.tile([128, TC, S], EXPTYPE, tag="expT", name=f"expT{hb}")
              for tcn in range(TC):
                  logits_exp(pair, tcn, (expT[ha], expT[hb]))
                  # interleave independent PE work to keep the PE fed:
                  # prior pair's AV halves + next pair's projections + V
                  if pair >= 1 and tcn % 2 == 1:
                      hprev = 2 * pair - 2 + (tcn // 4)
                      av_head_half(hprev, expT[hprev], (tcn // 2) % 2)
                  if pair == 0:
                      v_chunk(tcn)
                      if tcn in (1, 3):
                          proj_half(QT_sb, 1, tcn // 2, wq_sb, bq_sb)
                      elif tcn in (5, 7):
                          proj_half(KT_sb, 1, (tcn - 5) // 2, wk_sb, bk_sb)
                  elif pair < 3:
                      if tcn in (1, 3):
                          proj_half(QT_sb, pair + 1, tcn // 2, wq_sb, bq_sb)
                      elif tcn in (5, 7):
                          proj_half(KT_sb, pair + 1, (tcn - 5) // 2, wk_sb, bk_sb)

          wv_cm.__exit__(None, None, None)
          wqk_cm.__exit__(None, None, None)
          xT_cm.__exit__(None, None, None)

          wo_cm = tc.tile_pool(name="p_wo", bufs=1)
          p_wo = wo_cm.__enter__()
          wo_sb = p_wo.tile([128, 4, S], F32R, tag="wo")
          nc.sync.dma_start(out=wo_sb, in_=wo.ap().rearrange("(c p) n -> p c n", p=128))

          for h in (6, 7):
              for sh in range(SB):
                  av_head_half(h, expT[h], sh)

          attnT = late["attnT"]
          for st in range(TC):
              po = ps_lg.tile([128, 1024], F32, tag="lg", name=f"po{st}")
              for nh in range(SB):
                  for blk in range(4):
                      nc.tensor.matmul(
                          po[:, nh * 512:(nh + 1) * 512],
                          attnT[:, blk, st * 128:(st + 1) * 128],
                          wo_sb[:, blk, nh * 512:(nh + 1) * 512],
                          start=(blk == 0), stop=(blk == 3))
              o_sb = p_o.tile([128, 1024], F32, tag="o")
              nc.vector.tensor_copy(o_sb, po)
              nc.sync.dma_start(out=out[st * 128:(st + 1) * 128, :], in_=o_sb)

          for cm in (wo_cm, o_cm, nrm_cm, attn_cm, exp_cm, v_cm, qkt_cm,
                     misc_cm, avps_cm, lgps_cm, qkvps_cm):
              cm.__exit__(None, None, None)

    nc.compile()
    return nc


_NC = {}


def _get_nc(nrep=1):
    if nrep not in _NC:
        _NC[nrep] = _build(nrep)
    return _NC[nrep]


def kernel(x, mask, Wq, bq, Wk, bk, Wv, bv, Wo, bo, _trace=False):
    x = np.asarray(x, dtype=np.float32)
    mask = np.asarray(mask, dtype=np.float32)
    Wq, Wk, Wv, Wo = (np.asarray(w, dtype=np.float32) for w in (Wq, Wk, Wv, Wo))
    bq, bk, bv, bo = (np.asarray(b_, dtype=np.float32) for b_ in (bq, bk, bv, bo))

    nc = _get_nc()
    ones = np.ones((128, TC, HPG, 1), dtype=ml_dtypes.bfloat16)
    in_maps = []
    for c in range(NCORES):
        b, g = c // 2, c % 2
        sl = slice(g * GW, (g + 1) * GW)
        in_maps.append({
            "xT": np.ascontiguousarray(x[b].T),
            "wq": np.ascontiguousarray(Wq[:, sl]),
            "wk": np.ascontiguousarray(Wk[:, sl]),
            "wv": np.ascontiguousarray(Wv[:, sl]),
            "wo": np.ascontiguousarray(Wo[sl, :]),
            "mask1": np.ascontiguousarray(mask[b, 0, 0, :]),
            "bq1": np.ascontiguousarray(bq[sl]),
            "bk1": np.ascontiguousarray(bk[sl]),
            "bv1": np.ascontiguousarray(bv[sl]).reshape(1, GW),
            "ones": ones,
            "onecol": np.ones((1, HD), np.float32),
        })
    # First execution after NEFF load can race engine table initialization
    # (observed: garbage exp output on run 1 only). Warm up, then run.
    run_bass_kernel_spmd(nc, in_maps, core_ids=list(range(NCORES)))
    res = run_bass_kernel_spmd(
        nc, in_maps, core_ids=list(range(NCORES)), trace=_trace)
    kernel.last_results = res
    parts = [res.results[c]["out"] for c in range(NCORES)]
    return np.stack(
        [parts[2 * b] + parts[2 * b + 1] + bo for b in range(B)]
    ).astype(np.float32)



# revision 26
# speedup vs baseline: 1.0737x; 1.0737x over previous
"""Multi-head attention (B=4, S=1024, H=1024, 16 heads) on 8 TRN2 NeuronCores.

Sharding: core c = (batch b = c//2, head-group g = c%2). Each core computes
attention for its batch over 8 of the 16 heads (a 512-wide column slice of
the QKV projections) plus the matching row-slice of the output projection.
The two partial output projections per batch are summed on the host
(row-parallel tensor-parallel unshard), where the bias bo is also added.

On-core dataflow (matmuls in f32r except the attention-weight matmul in
bf16; psum accumulation is fp32 throughout):
  QT[hd,s] = Wq_g^T x^T (+bq)   KT likewise (+bk)    V[t,hd] = x Wv_g (+bv)
  logitsT[t,s] per head: contraction over d=64; two heads packed in the PE
                         via tile_position row groups
  expT = exp(logitsT/8 + mask*NEG_INF)   (mask enters as per-partition bias)
  AVT[d,s] & colsum = [V_h | 1]^T @ expT (ones column makes psum row 64 the
                                          softmax denominator)
  attnT = AVT * (1/colsum broadcast)     (broadcast via k=1 ones matmul)
  out[s,n] += attnT-chunk^T @ Wo_g       (partial; host sums core pairs)
"""
import sys

sys.path.insert(0, "/opt/trn_rl_repo")

import ml_dtypes
import numpy as np

import concourse.bass as bass
import concourse.mybir as mybir
import concourse.tile as tile
from concourse import bacc
from concourse.bass_utils import run_bass_kernel_spmd

F32 = mybir.dt.float32
F32R = mybir.dt.float32r
BF16 = mybir.dt.bfloat16
EXPTYPE = BF16

B, S, H = 4, 1024, 1024
NH, HD = 16, 64
HPG = 8            # heads per group (per core)
GW = HPG * HD      # 512: group width
NEG_INF = -2.0 ** 32
NCORES = 8
HC = H // 128      # 8 contraction chunks over hidden
TC = S // 128      # 8 chunks over key positions t
SB = S // 512      # 2 halves of the s (query) axis

Exp = mybir.ActivationFunctionType.Exp


def _build(nrep=1):
    nc = bacc.Bacc("TRN2", target_bir_lowering=False, debug=False)

    xT = nc.dram_tensor("xT", [H, S], F32R, kind="ExternalInput")
    wq = nc.dram_tensor("wq", [H, GW], F32R, kind="ExternalInput")
    wk = nc.dram_tensor("wk", [H, GW], F32R, kind="ExternalInput")
    wv = nc.dram_tensor("wv", [H, GW], F32R, kind="ExternalInput")
    wo = nc.dram_tensor("wo", [GW, S], F32R, kind="ExternalInput")
    mask1 = nc.dram_tensor("mask1", [S], F32, kind="ExternalInput")
    bq1 = nc.dram_tensor("bq1", [GW], F32, kind="ExternalInput")
    bk1 = nc.dram_tensor("bk1", [GW], F32, kind="ExternalInput")
    bv1 = nc.dram_tensor("bv1", [1, GW], F32, kind="ExternalInput")
    ones = nc.dram_tensor("ones", [128, TC, HPG, 1], EXPTYPE, kind="ExternalInput")
    onecol = nc.dram_tensor("onecol", [1, HD], F32R, kind="ExternalInput")
    out = nc.dram_tensor("out", [S, H], F32, kind="ExternalOutput")

    with tile.TileContext(nc, pool_alloc_mode="stack") as tc:
      for _rep in range(nrep):
          # Pool releases must be LIFO, so the three big input pools
          # (xT/wqk/wv) are created LAST: they release mid-kernel (stack
          # rewinds) and p_wo then reuses their space.
          misc_cm = tc.tile_pool(name="misc", bufs=1); misc = misc_cm.__enter__()
          qkt_cm = tc.tile_pool(name="p_qkt", bufs=1); p_qkt = qkt_cm.__enter__()
          v_cm = tc.tile_pool(name="p_v", bufs=1); p_v = v_cm.__enter__()
          exp_cm = tc.tile_pool(name="p_exp", bufs=3); p_exp = exp_cm.__enter__()
          attn_cm = tc.tile_pool(name="p_attn", bufs=1)
          p_attn = attn_cm.__enter__()
          nrm_cm = tc.tile_pool(name="p_nrm", bufs=2); p_nrm = nrm_cm.__enter__()
          o_cm = tc.tile_pool(name="p_o", bufs=3); p_o = o_cm.__enter__()
          xT_cm = tc.tile_pool(name="p_xT", bufs=1); p_xT = xT_cm.__enter__()
          wqk_cm = tc.tile_pool(name="p_wqk", bufs=1); p_wqk = wqk_cm.__enter__()
          wv_cm = tc.tile_pool(name="p_wv", bufs=1); p_wv = wv_cm.__enter__()
          late = {"p_nrm": p_nrm}
          qkvps_cm = tc.tile_pool(name="ps_qkv", bufs=2, space="PSUM")
          ps_qkv = qkvps_cm.__enter__()
          lgps_cm = tc.tile_pool(name="ps_lg", bufs=2, space="PSUM")
          ps_lg = lgps_cm.__enter__()
          avps_cm = tc.tile_pool(name="ps_av", bufs=2, space="PSUM")
          ps_av = avps_cm.__enter__()

          # ---- tiny const DMAs first (cheap; they gate exp and copies) ----
          maskb = misc.tile([128, TC], F32, tag="maskb")
          mraw = misc.tile([128, TC], F32, tag="mraw")
          nc.sync.dma_start(out=mraw, in_=mask1.ap().rearrange("(c p) -> p c", p=128))
          nc.vector.tensor_scalar_mul(maskb, mraw, NEG_INF)
          bq_sb = misc.tile([128, 4], F32, tag="bq")
          bk_sb = misc.tile([128, 4], F32, tag="bk")
          nc.sync.dma_start(out=bq_sb, in_=bq1.ap().rearrange("(c p) -> p c", p=128))
          nc.sync.dma_start(out=bk_sb, in_=bk1.ap().rearrange("(c p) -> p c", p=128))
          bv_bc = misc.tile([128, GW], F32, tag="bv")
          nc.sync.dma_start(out=bv_bc, in_=bv1[0:1, :].to_broadcast((128, GW)))
          onecol_sb = misc.tile([1, HD], F32R, tag="onecol")
          nc.sync.dma_start(out=onecol_sb, in_=onecol[:, :])

          # ---- big input loads first: first QT matmul needs xT + wq blk0,
          # so those DMAs get top scheduler priority; weights split per block.
          xT_sb = p_xT.tile([128, HC, S], F32R, tag="xT")
          wq_sb = p_wqk.tile([128, HC, GW], F32R, tag="wq")
          wk_sb = p_wqk.tile([128, HC, GW], F32R, tag="wk")
          wv_sb = p_wv.tile([128, HC, GW], F32R, tag="wv")
          wq_r = wq.ap().rearrange("(c p) m -> p c m", p=128)
          wk_r = wk.ap().rearrange("(c p) m -> p c m", p=128)
          # operands of the very first matmuls first: wq blk0, then xT sh0
          nc.sync.dma_start(out=wq_sb[:, :, 0:128], in_=wq_r[:, :, 0:128])
          for c in range(HC):
              nc.sync.dma_start(out=xT_sb[:, c, 0:512], in_=xT[c * 128:(c + 1) * 128, 0:512])
          nc.sync.dma_start(out=wk_sb[:, :, 0:128], in_=wk_r[:, :, 0:128])
          for c in range(HC):
              nc.sync.dma_start(out=xT_sb[:, c, 512:1024], in_=xT[c * 128:(c + 1) * 128, 512:1024])
          for blk in range(1, 4):
              bs = slice(blk * 128, (blk + 1) * 128)
              nc.sync.dma_start(out=wq_sb[:, :, bs], in_=wq_r[:, :, bs])
          for blk in range(1, 4):
              bs = slice(blk * 128, (blk + 1) * 128)
              nc.sync.dma_start(out=wk_sb[:, :, bs], in_=wk_r[:, :, bs])
          nc.sync.dma_start(out=wv_sb, in_=wv.ap().rearrange("(c p) m -> p c m", p=128))


          QT_sb = p_qkt.tile([128, 4, S], F32R, tag="QT")
          KT_sb = p_qkt.tile([128, 4, S], F32R, tag="KT")
          V_sb = p_v.tile([128, TC, HPG, HD + 1], EXPTYPE, tag="V")
          nc.sync.dma_start(out=V_sb[:, :, :, HD:HD + 1], in_=ones.ap())
          late["attnT"] = p_attn.tile([128, 4, S], F32R, tag="attnT", name="attnT")

          def proj_half(dst, blk, sh, w_sb, b_sb):
              """dst[:, blk, sh-half] (+bias) = block of Wg^T x^T."""
              ps = ps_qkv.tile([128, 512], F32, tag="mm512")
              for c in range(HC):
                  nc.tensor.matmul(
                      ps, w_sb[:, c, blk * 128:(blk + 1) * 128],
                      xT_sb[:, c, sh * 512:(sh + 1) * 512],
                      start=(c == 0), stop=(c == HC - 1))
              nc.vector.tensor_scalar_add(
                  dst[:, blk, sh * 512:(sh + 1) * 512], ps, b_sb[:, blk:blk + 1])

          def v_chunk(tcn):
              """V_sb[:, tcn, :, 0:64] (+bv) = rows 128*tcn.. of x Wv_g."""
              ps = ps_qkv.tile([128, 512], F32, tag="mm512")
              for c in range(HC):
                  nc.tensor.matmul(
                      ps, xT_sb[:, c, tcn * 128:(tcn + 1) * 128], wv_sb[:, c, :],
                      start=(c == 0), stop=(c == HC - 1))
              nc.vector.tensor_add(
                  V_sb[:, tcn, :, 0:HD],
                  ps.rearrange("p (h d) -> p h d", h=HPG),
                  bv_bc.rearrange("p (h d) -> p h d", h=HPG))

          def logits_exp(pair, tcn, exp_dsts, split=False):
              """Packed pair of d=64 logit matmuls + exp for chunk tcn.
              split=True: one exp per s-half, so the first exps don't wait
              for the late-arriving second half of xT."""
              for i, (off, tp) in enumerate(((0, (0, 0)), (64, (64, 0)))):
                  lg = ps_lg.tile([128, 1024], F32, tag="lg")
                  for sh in range(SB):
                      nc.tensor.matmul(
                          lg[:, sh * 512:(sh + 1) * 512],
                          KT_sb[off:off + 64, pair, tcn * 128:(tcn + 1) * 128],
                          QT_sb[off:off + 64, pair, sh * 512:(sh + 1) * 512],
                          start=True, stop=True, tile_position=tp)
                      if split:
                          nc.scalar.activation(
                              out=exp_dsts[i][:, tcn, sh * 512:(sh + 1) * 512],
                              in_=lg[:, sh * 512:(sh + 1) * 512], func=Exp,
                              bias=maskb[:, tcn:tcn + 1], scale=0.125)
                  if not split:
                      nc.scalar.activation(
                          out=exp_dsts[i][:, tcn, :], in_=lg, func=Exp,
                          bias=maskb[:, tcn:tcn + 1], scale=0.125)

          def av_head_half(h, expT_h, sh):
              """attnT rows for head h, s-half sh = normalized V_h^T @ expT_h."""
              off = (h % 2) * 64
              if True:
                  pav = ps_av.tile([HD + 1, 512], F32, tag="av",
                                    name=f"pav{h}_{sh}")
                  for tcn in range(TC):
                      nc.tensor.matmul(
                          pav, V_sb[:, tcn, h, :],
                          expT_h[:, tcn, sh * 512:(sh + 1) * 512],
                          start=(tcn == 0), stop=(tcn == TC - 1))
                  recip = late["p_nrm"].tile([1, 512], F32R, tag="recip")
                  with nc.allow_low_precision(reason="softmax denom recip to f32r"):
                      nc.vector.reciprocal(recip, pav[HD:HD + 1, :])
                  bps = ps_qkv.tile([HD, 512], F32, tag="mm512",
                                    name=f"bps{h}_{sh}")
                  nc.tensor.matmul(bps, onecol_sb, recip, start=True, stop=True)
                  bcast = late["p_nrm"].tile([HD, 512], F32, tag="bcast")
                  nc.vector.tensor_copy(bcast, bps)
                  nc.vector.tensor_mul(
                      late["attnT"][off:off + HD, h // 2, sh * 512:(sh + 1) * 512],
                      pav[0:HD, :], bcast)

          # ---------------- emission ----------------
          for sh in range(SB):
              proj_half(QT_sb, 0, sh, wq_sb, bq_sb)
          for sh in range(SB):
              proj_half(KT_sb, 0, sh, wk_sb, bk_sb)

          expT = {}
          for pair in range(4):
              ha, hb = 2 * pair, 2 * pair + 1
              if pair >= 1:
                  # free head 2p-2's expT slot ASAP: its exps finished during
                  # the previous loop, and the next pair's second tile waits
                  # on this slot
                  av_head_half(2 * pair - 2, expT[2 * pair - 2], 0)
              expT[ha] = p_exp.tile([128, TC, S], EXPTYPE, tag="expT", name=f"expT{ha}")
              expT[hb] = p_exp.tile([128, TC, S], EXPTYPE, tag="expT", name=f"expT{hb}")
              for tcn in range(TC):
                  # emit independent PE filler BEFORE logits: the logits
                  # matmul waits on its psum slot (paced by ACT exp), and the
                  # PE executes in order, so filler placed after it would
                  # head-of-line block
                  if pair >= 1 and tcn == 1:
                      av_head_half(2 * pair - 2, expT[2 * pair - 2], 1)
                  if pair >= 1 and tcn in (2, 4):
                      av_head_half(2 * pair - 1, expT[2 * pair - 1], tcn // 2 - 1)
                  if pair == 0:
                      if tcn > 0:
                          v_chunk(tcn)
                      if tcn in (1, 3):
                          proj_half(QT_sb, 1, tcn // 2, wq_sb, bq_sb)
                      elif tcn in (5, 7):
                          proj_half(KT_sb, 1, (tcn - 5) // 2, wk_sb, bk_sb)
                  elif pair < 3:
                      if tcn in (1, 3):
                          proj_half(QT_sb, pair + 1, tcn // 2, wq_sb, bq_sb)
                      elif tcn in (5, 7):
                          proj_half(KT_sb, pair + 1, (tcn - 5) // 2, wk_sb, bk_sb)
                  logits_exp(pair, tcn, (expT[ha], expT[hb]))
                  if pair == 0 and tcn == 0:
                      v_chunk(0)

          wv_cm.__exit__(None, None, None)
          wqk_cm.__exit__(None, None, None)
          xT_cm.__exit__(None, None, None)

          wo_cm = tc.tile_pool(name="p_wo", bufs=1)
          p_wo = wo_cm.__enter__()
          wo_sb = p_wo.tile([128, 4, S], F32R, tag="wo")
          nc.sync.dma_start(out=wo_sb, in_=wo.ap().rearrange("(c p) n -> p c n", p=128))

          for h in (6, 7):
              for sh in range(SB):
                  av_head_half(h, expT[h], sh)

          attnT = late["attnT"]
          for st in range(TC):
              po = ps_lg.tile([128, 1024], F32, tag="lg", name=f"po{st}")
              for nh in range(SB):
                  for blk in range(4):
                      nc.tensor.matmul(
                          po[:, nh * 512:(nh + 1) * 512],
                          attnT[:, blk, st * 128:(st + 1) * 128],
                          wo_sb[:, blk, nh * 512:(nh + 1) * 512],
                          start=(blk == 0), stop=(blk == 3))
              o_sb = p_o.tile([128, 1024], F32, tag="o")
              nc.vector.tensor_copy(o_sb, po)
              nc.sync.dma_start(out=out[st * 128:(st + 1) * 128, :], in_=o_sb)

          for cm in (wo_cm, o_cm, nrm_cm, attn_cm, exp_cm, v_cm, qkt_cm,
                     misc_cm, avps_cm, lgps_cm, qkvps_cm):
              cm.__exit__(None, None, None)

    nc.compile()
    return nc


_NC = {}


def _get_nc(nrep=1):
    if nrep not in _NC:
        _NC[nrep] = _build(nrep)
    return _NC[nrep]


def kernel(x, mask, Wq, bq, Wk, bk, Wv, bv, Wo, bo, _trace=False):
    x = np.asarray(x, dtype=np.float32)
    mask = np.asarray(mask, dtype=np.float32)
    Wq, Wk, Wv, Wo = (np.asarray(w, dtype=np.float32) for w in (Wq, Wk, Wv, Wo))
    bq, bk, bv, bo = (np.asarray(b_, dtype=np.float32) for b_ in (bq, bk, bv, bo))

    nc = _get_nc()
    ones = np.ones((128, TC, HPG, 1), dtype=ml_dtypes.bfloat16)
    in_maps = []
    for c in range(NCORES):
        b, g = c // 2, c % 2
        sl = slice(g * GW, (g + 1) * GW)
        in_maps.append({
            "xT": np.ascontiguousarray(x[b].T),
            "wq": np.ascontiguousarray(Wq[:, sl]),
            "wk": np.ascontiguousarray(Wk[:, sl]),
            "wv": np.ascontiguousarray(Wv[:, sl]),
            "wo": np.ascontiguousarray(Wo[sl, :]),
            "mask1": np.ascontiguousarray(mask[b, 0, 0, :]),
            "bq1": np.ascontiguousarray(bq[sl]),
            "bk1": np.ascontiguousarray(bk[sl]),
            "bv1": np.ascontiguousarray(bv[sl]).reshape(1, GW),
            "ones": ones,
            "onecol": np.ones((1, HD), np.float32),
        })
    # First execution after NEFF load can race engine table initialization
    # (observed: garbage exp output on run 1 only). Warm up, then run.
    run_bass_kernel_spmd(nc, in_maps, core_ids=list(range(NCORES)))
    res = run_bass_kernel_spmd(
        nc, in_maps, core_ids=list(range(NCORES)), trace=_trace)
    kernel.last_results = res
    parts = [res.results[c]["out"] for c in range(NCORES)]
    return np.stack(
        [parts[2 * b] + parts[2 * b + 1] + bo for b in range(B)]
    ).astype(np.float32)



# revision 32
# speedup vs baseline: 1.0760x; 1.0021x over previous
"""Multi-head attention (B=4, S=1024, H=1024, 16 heads) on 8 TRN2 NeuronCores.

Sharding: core c = (batch b = c//2, head-group g = c%2). Each core computes
attention for its batch over 8 of the 16 heads (a 512-wide column slice of
the QKV projections) plus the matching row-slice of the output projection.
The two partial output projections per batch are summed on the host
(row-parallel tensor-parallel unshard), where the bias bo is also added.

On-core dataflow (matmuls in f32r except the attention-weight matmul in
bf16; psum accumulation is fp32 throughout):
  QT[hd,s] = Wq_g^T x^T (+bq)   KT likewise (+bk)    V[t,hd] = x Wv_g (+bv)
  logitsT[t,s] per head: contraction over d=64; two heads packed in the PE
                         via tile_position row groups
  expT = exp(logitsT/8 + mask*NEG_INF)   (mask enters as per-partition bias)
  AVT[d,s] & colsum = [V_h | 1]^T @ expT (ones column makes psum row 64 the
                                          softmax denominator)
  attnT = AVT * (1/colsum broadcast)     (broadcast via k=1 ones matmul)
  out[s,n] += attnT-chunk^T @ Wo_g       (partial; host sums core pairs)
"""
import sys

sys.path.insert(0, "/opt/trn_rl_repo")

import ml_dtypes
import numpy as np

import concourse.bass as bass
import concourse.mybir as mybir
import concourse.tile as tile
from concourse import bacc
from concourse.bass_utils import run_bass_kernel_spmd

F32 = mybir.dt.float32
F32R = mybir.dt.float32r
BF16 = mybir.dt.bfloat16
EXPTYPE = BF16

B, S, H = 4, 1024, 1024
NH, HD = 16, 64
HPG = 8            # heads per group (per core)
GW = HPG * HD      # 512: group width
NEG_INF = -2.0 ** 32
NCORES = 8
HC = H // 128      # 8 contraction chunks over hidden
TC = S // 128      # 8 chunks over key positions t
SB = S // 512      # 2 halves of the s (query) axis

Exp = mybir.ActivationFunctionType.Exp


def _build(nrep=1):
    nc = bacc.Bacc("TRN2", target_bir_lowering=False, debug=False)

    xT = nc.dram_tensor("xT", [H, S], F32R, kind="ExternalInput")
    wq = nc.dram_tensor("wq", [H, GW], F32R, kind="ExternalInput")
    wk = nc.dram_tensor("wk", [H, GW], F32R, kind="ExternalInput")
    wv = nc.dram_tensor("wv", [H, GW], F32R, kind="ExternalInput")
    wo = nc.dram_tensor("wo", [GW, S], F32R, kind="ExternalInput")
    mask1 = nc.dram_tensor("mask1", [S], F32, kind="ExternalInput")
    bq1 = nc.dram_tensor("bq1", [GW], F32, kind="ExternalInput")
    bk1 = nc.dram_tensor("bk1", [GW], F32, kind="ExternalInput")
    bv1 = nc.dram_tensor("bv1", [1, GW], F32, kind="ExternalInput")
    ones = nc.dram_tensor("ones", [128, TC, HPG, 1], EXPTYPE, kind="ExternalInput")
    onecol = nc.dram_tensor("onecol", [1, HD], F32R, kind="ExternalInput")
    out = nc.dram_tensor("out", [S, H], F32, kind="ExternalOutput")

    with tile.TileContext(nc, pool_alloc_mode="stack") as tc:
      for _rep in range(nrep):
          # Pool releases must be LIFO, so the three big input pools
          # (xT/wqk/wv) are created LAST: they release mid-kernel (stack
          # rewinds) and p_wo then reuses their space.
          misc_cm = tc.tile_pool(name="misc", bufs=1); misc = misc_cm.__enter__()
          qkt_cm = tc.tile_pool(name="p_qkt", bufs=1); p_qkt = qkt_cm.__enter__()
          v_cm = tc.tile_pool(name="p_v", bufs=1); p_v = v_cm.__enter__()
          exp_cm = tc.tile_pool(name="p_exp", bufs=3); p_exp = exp_cm.__enter__()
          attn_cm = tc.tile_pool(name="p_attn", bufs=1)
          p_attn = attn_cm.__enter__()
          nrm_cm = tc.tile_pool(name="p_nrm", bufs=2); p_nrm = nrm_cm.__enter__()
          o_cm = tc.tile_pool(name="p_o", bufs=3); p_o = o_cm.__enter__()
          xT_cm = tc.tile_pool(name="p_xT", bufs=1); p_xT = xT_cm.__enter__()
          wqk_cm = tc.tile_pool(name="p_wqk", bufs=1); p_wqk = wqk_cm.__enter__()
          wv_cm = tc.tile_pool(name="p_wv", bufs=1); p_wv = wv_cm.__enter__()
          late = {"p_nrm": p_nrm}
          qkvps_cm = tc.tile_pool(name="ps_qkv", bufs=2, space="PSUM")
          ps_qkv = qkvps_cm.__enter__()
          lgps_cm = tc.tile_pool(name="ps_lg", bufs=2, space="PSUM")
          ps_lg = lgps_cm.__enter__()
          avps_cm = tc.tile_pool(name="ps_av", bufs=2, space="PSUM")
          ps_av = avps_cm.__enter__()

          # ---- tiny const DMAs first (cheap; they gate exp and copies) ----
          maskb = misc.tile([128, TC], F32, tag="maskb")
          mraw = misc.tile([128, TC], F32, tag="mraw")
          nc.sync.dma_start(out=mraw, in_=mask1.ap().rearrange("(c p) -> p c", p=128))
          nc.vector.tensor_scalar_mul(maskb, mraw, NEG_INF)
          bq_sb = misc.tile([128, 4], F32, tag="bq")
          bk_sb = misc.tile([128, 4], F32, tag="bk")
          nc.sync.dma_start(out=bq_sb, in_=bq1.ap().rearrange("(c p) -> p c", p=128))
          nc.sync.dma_start(out=bk_sb, in_=bk1.ap().rearrange("(c p) -> p c", p=128))
          bv_bc = misc.tile([128, GW], F32, tag="bv")
          nc.sync.dma_start(out=bv_bc, in_=bv1[0:1, :].to_broadcast((128, GW)))
          onecol_sb = misc.tile([1, HD], F32R, tag="onecol")
          nc.sync.dma_start(out=onecol_sb, in_=onecol[:, :])

          # ---- big input loads first: first QT matmul needs xT + wq blk0,
          # so those DMAs get top scheduler priority; weights split per block.
          xT_sb = p_xT.tile([128, HC, S], F32R, tag="xT")
          wq_sb = p_wqk.tile([128, HC, GW], F32R, tag="wq")
          wk_sb = p_wqk.tile([128, HC, GW], F32R, tag="wk")
          wv_sb = p_wv.tile([128, HC, GW], F32R, tag="wv")
          wq_r = wq.ap().rearrange("(c p) m -> p c m", p=128)
          wk_r = wk.ap().rearrange("(c p) m -> p c m", p=128)
          # operands of the very first matmuls first: wq blk0, then xT sh0
          nc.sync.dma_start(out=wq_sb[:, :, 0:128], in_=wq_r[:, :, 0:128])
          for c in range(HC):
              nc.sync.dma_start(out=xT_sb[:, c, 0:512], in_=xT[c * 128:(c + 1) * 128, 0:512])
          nc.sync.dma_start(out=wk_sb[:, :, 0:128], in_=wk_r[:, :, 0:128])
          for c in range(HC):
              nc.sync.dma_start(out=xT_sb[:, c, 512:1024], in_=xT[c * 128:(c + 1) * 128, 512:1024])
          for blk in range(1, 4):
              bs = slice(blk * 128, (blk + 1) * 128)
              nc.sync.dma_start(out=wq_sb[:, :, bs], in_=wq_r[:, :, bs])
          for blk in range(1, 4):
              bs = slice(blk * 128, (blk + 1) * 128)
              nc.sync.dma_start(out=wk_sb[:, :, bs], in_=wk_r[:, :, bs])
          nc.sync.dma_start(out=wv_sb, in_=wv.ap().rearrange("(c p) m -> p c m", p=128))


          QT_sb = p_qkt.tile([128, 4, S], F32R, tag="QT")
          KT_sb = p_qkt.tile([128, 4, S], F32R, tag="KT")
          V_sb = p_v.tile([128, TC, HPG, HD + 1], EXPTYPE, tag="V")
          nc.sync.dma_start(out=V_sb[:, :, :, HD:HD + 1], in_=ones.ap())
          late["attnT"] = p_attn.tile([128, 4, S], F32R, tag="attnT", name="attnT")

          def proj_half(dst, blk, sh, w_sb, b_sb):
              """dst[:, blk, sh-half] (+bias) = block of Wg^T x^T."""
              ps = ps_qkv.tile([128, 512], F32, tag="mm512")
              for c in range(HC):
                  nc.tensor.matmul(
                      ps, w_sb[:, c, blk * 128:(blk + 1) * 128],
                      xT_sb[:, c, sh * 512:(sh + 1) * 512],
                      start=(c == 0), stop=(c == HC - 1))
              nc.vector.tensor_scalar_add(
                  dst[:, blk, sh * 512:(sh + 1) * 512], ps, b_sb[:, blk:blk + 1])

          def v_chunk(tcn):
              """V_sb[:, tcn, :, 0:64] (+bv) = rows 128*tcn.. of x Wv_g."""
              ps = ps_qkv.tile([128, 512], F32, tag="mm512")
              for c in range(HC):
                  nc.tensor.matmul(
                      ps, xT_sb[:, c, tcn * 128:(tcn + 1) * 128], wv_sb[:, c, :],
                      start=(c == 0), stop=(c == HC - 1))
              nc.vector.tensor_add(
                  V_sb[:, tcn, :, 0:HD],
                  ps.rearrange("p (h d) -> p h d", h=HPG),
                  bv_bc.rearrange("p (h d) -> p h d", h=HPG))

          def logits_exp(pair, tcn, exp_dsts, split=False):
              """Packed pair of d=64 logit matmuls + exp for chunk tcn.
              split=True: one exp per s-half, so the first exps don't wait
              for the late-arriving second half of xT."""
              for i, (off, tp) in enumerate(((0, (0, 0)), (64, (64, 0)))):
                  lg = ps_lg.tile([128, 1024], F32, tag="lg")
                  for sh in range(SB):
                      nc.tensor.matmul(
                          lg[:, sh * 512:(sh + 1) * 512],
                          KT_sb[off:off + 64, pair, tcn * 128:(tcn + 1) * 128],
                          QT_sb[off:off + 64, pair, sh * 512:(sh + 1) * 512],
                          start=True, stop=True, tile_position=tp)
                      if split:
                          nc.scalar.activation(
                              out=exp_dsts[i][:, tcn, sh * 512:(sh + 1) * 512],
                              in_=lg[:, sh * 512:(sh + 1) * 512], func=Exp,
                              bias=maskb[:, tcn:tcn + 1], scale=0.125)
                  if not split:
                      nc.scalar.activation(
                          out=exp_dsts[i][:, tcn, :], in_=lg, func=Exp,
                          bias=maskb[:, tcn:tcn + 1], scale=0.125)

          def av_head_half(h, expT_h, sh):
              """attnT rows for head h, s-half sh = normalized V_h^T @ expT_h."""
              off = (h % 2) * 64
              if True:
                  pav = ps_av.tile([HD + 1, 512], F32, tag="av",
                                    name=f"pav{h}_{sh}")
                  for tcn in range(TC):
                      nc.tensor.matmul(
                          pav, V_sb[:, tcn, h, :],
                          expT_h[:, tcn, sh * 512:(sh + 1) * 512],
                          start=(tcn == 0), stop=(tcn == TC - 1))
                  recip = late["p_nrm"].tile([1, 512], F32R, tag="recip")
                  with nc.allow_low_precision(reason="softmax denom recip to f32r"):
                      nc.vector.reciprocal(recip, pav[HD:HD + 1, :])
                  bps = ps_qkv.tile([HD, 512], F32, tag="mm512",
                                    name=f"bps{h}_{sh}")
                  nc.tensor.matmul(bps, onecol_sb, recip, start=True, stop=True)
                  bcast = late["p_nrm"].tile([HD, 512], F32, tag="bcast")
                  nc.vector.tensor_copy(bcast, bps)
                  nc.vector.tensor_mul(
                      late["attnT"][off:off + HD, h // 2, sh * 512:(sh + 1) * 512],
                      pav[0:HD, :], bcast)

          # ---------------- emission ----------------
          for sh in range(SB):
              proj_half(QT_sb, 0, sh, wq_sb, bq_sb)
          for sh in range(SB):
              proj_half(KT_sb, 0, sh, wk_sb, bk_sb)

          expT = {}
          for pair in range(4):
              ha, hb = 2 * pair, 2 * pair + 1
              if pair >= 1:
                  # free head 2p-2's expT slot ASAP: its exps finished during
                  # the previous loop, and the next pair's second tile waits
                  # on this slot
                  av_head_half(2 * pair - 2, expT[2 * pair - 2], 0)
              expT[ha] = p_exp.tile([128, TC, S], EXPTYPE, tag="expT", name=f"expT{ha}")
              expT[hb] = p_exp.tile([128, TC, S], EXPTYPE, tag="expT", name=f"expT{hb}")
              for tcn in range(TC):
                  # emit independent PE filler BEFORE logits: the logits
                  # matmul waits on its psum slot (paced by ACT exp), and the
                  # PE executes in order, so filler placed after it would
                  # head-of-line block
                  if pair >= 1 and tcn == 1:
                      av_head_half(2 * pair - 2, expT[2 * pair - 2], 1)
                  if pair >= 1 and tcn in (2, 4):
                      av_head_half(2 * pair - 1, expT[2 * pair - 1], tcn // 2 - 1)
                  if pair == 0:
                      if tcn > 0:
                          v_chunk(tcn)
                      if tcn in (1, 3):
                          proj_half(QT_sb, 1, tcn // 2, wq_sb, bq_sb)
                      elif tcn in (5, 7):
                          proj_half(KT_sb, 1, (tcn - 5) // 2, wk_sb, bk_sb)
                  elif pair < 3:
                      if tcn in (1, 3):
                          proj_half(QT_sb, pair + 1, tcn // 2, wq_sb, bq_sb)
                      elif tcn in (5, 7):
                          proj_half(KT_sb, pair + 1, (tcn - 5) // 2, wk_sb, bk_sb)
                  logits_exp(pair, tcn, (expT[ha], expT[hb]))
                  if pair == 0 and tcn == 0:
                      v_chunk(0)

          wv_cm.__exit__(None, None, None)
          wqk_cm.__exit__(None, None, None)
          xT_cm.__exit__(None, None, None)

          wo_cm = tc.tile_pool(name="p_wo", bufs=1)
          p_wo = wo_cm.__enter__()
          wo_sb = p_wo.tile([128, 4, S], F32R, tag="wo")
          nc.sync.dma_start(out=wo_sb, in_=wo.ap().rearrange("(c p) n -> p c n", p=128))

          for h in (6, 7):
              for sh in range(SB):
                  av_head_half(h, expT[h], sh)

          attnT = late["attnT"]
          for st in range(TC):
              po = ps_lg.tile([128, 1024], F32, tag="lg", name=f"po{st}")
              for nh in range(SB):
                  for blk in range(4):
                      nc.tensor.matmul(
                          po[:, nh * 512:(nh + 1) * 512],
                          attnT[:, blk, st * 128:(st + 1) * 128],
                          wo_sb[:, blk, nh * 512:(nh + 1) * 512],
                          start=(blk == 0), stop=(blk == 3))
              o_sb = p_o.tile([128, 1024], F32, tag="o")
              nc.scalar.copy(o_sb, po)
              nc.sync.dma_start(out=out[st * 128:(st + 1) * 128, :], in_=o_sb)

          for cm in (wo_cm, o_cm, nrm_cm, attn_cm, exp_cm, v_cm, qkt_cm,
                     misc_cm, avps_cm, lgps_cm, qkvps_cm):
              cm.__exit__(None, None, None)

    nc.compile()
    return nc


_NC = {}


def _get_nc(nrep=1):
    if nrep not in _NC:
        _NC[nrep] = _build(nrep)
    return _NC[nrep]


def kernel(x, mask, Wq, bq, Wk, bk, Wv, bv, Wo, bo, _trace=False):
    x = np.asarray(x, dtype=np.float32)
    mask = np.asarray(mask, dtype=np.float32)
    Wq, Wk, Wv, Wo = (np.asarray(w, dtype=np.float32) for w in (Wq, Wk, Wv, Wo))
    bq, bk, bv, bo = (np.asarray(b_, dtype=np.float32) for b_ in (bq, bk, bv, bo))

    nc = _get_nc()
    ones = np.ones((128, TC, HPG, 1), dtype=ml_dtypes.bfloat16)
    in_maps = []
    for c in range(NCORES):
        b, g = c // 2, c % 2
        sl = slice(g * GW, (g + 1) * GW)
        in_maps.append({
            "xT": np.ascontiguousarray(x[b].T),
            "wq": np.ascontiguousarray(Wq[:, sl]),
            "wk": np.ascontiguousarray(Wk[:, sl]),
            "wv": np.ascontiguousarray(Wv[:, sl]),
            "wo": np.ascontiguousarray(Wo[sl, :]),
            "mask1": np.ascontiguousarray(mask[b, 0, 0, :]),
            "bq1": np.ascontiguousarray(bq[sl]),
            "bk1": np.ascontiguousarray(bk[sl]),
            "bv1": np.ascontiguousarray(bv[sl]).reshape(1, GW),
            "ones": ones,
            "onecol": np.ones((1, HD), np.float32),
        })
    # First execution after NEFF load can race engine table initialization
    # (observed: garbage exp output on run 1 only). Warm up, then run.
    run_bass_kernel_spmd(nc, in_maps, core_ids=list(range(NCORES)))
    res = run_bass_kernel_spmd(
        nc, in_maps, core_ids=list(range(NCORES)), trace=_trace)
    kernel.last_results = res
    parts = [res.results[c]["out"] for c in range(NCORES)]
    return np.stack(
        [parts[2 * b] + parts[2 * b + 1] + bo for b in range(B)]
    ).astype(np.float32)



# revision 33
# speedup vs baseline: 1.0800x; 1.0037x over previous
"""Multi-head attention (B=4, S=1024, H=1024, 16 heads) on 8 TRN2 NeuronCores.

Sharding: core c = (batch b = c//2, head-group g = c%2). Each core computes
attention for its batch over 8 of the 16 heads (a 512-wide column slice of
the QKV projections) plus the matching row-slice of the output projection.
The two partial output projections per batch are summed on the host
(row-parallel tensor-parallel unshard), where the bias bo is also added.

On-core dataflow (matmuls in f32r except the attention-weight matmul in
bf16; psum accumulation is fp32 throughout):
  QT[hd,s] = Wq_g^T x^T (+bq)   KT likewise (+bk)    V[t,hd] = x Wv_g (+bv)
  logitsT[t,s] per head: contraction over d=64; two heads packed in the PE
                         via tile_position row groups
  expT = exp(logitsT/8 + mask*NEG_INF)   (mask enters as per-partition bias)
  AVT[d,s] & colsum = [V_h | 1]^T @ expT (ones column makes psum row 64 the
                                          softmax denominator)
  attnT = AVT * (1/colsum broadcast)     (broadcast via k=1 ones matmul)
  out[s,n] += attnT-chunk^T @ Wo_g       (partial; host sums core pairs)
"""
import sys

sys.path.insert(0, "/opt/trn_rl_repo")

import ml_dtypes
import numpy as np

import concourse.bass as bass
import concourse.mybir as mybir
import concourse.tile as tile
from concourse import bacc
from concourse.bass_utils import run_bass_kernel_spmd

F32 = mybir.dt.float32
F32R = mybir.dt.float32r
BF16 = mybir.dt.bfloat16
EXPTYPE = BF16

B, S, H = 4, 1024, 1024
NH, HD = 16, 64
HPG = 8            # heads per group (per core)
GW = HPG * HD      # 512: group width
NEG_INF = -2.0 ** 32
NCORES = 8
HC = H // 128      # 8 contraction chunks over hidden
TC = S // 128      # 8 chunks over key positions t
SB = S // 512      # 2 halves of the s (query) axis

Exp = mybir.ActivationFunctionType.Exp


def _build(nrep=1):
    nc = bacc.Bacc("TRN2", target_bir_lowering=False, debug=False)

    xT = nc.dram_tensor("xT", [H, S], F32R, kind="ExternalInput")
    wq = nc.dram_tensor("wq", [H, GW], F32R, kind="ExternalInput")
    wk = nc.dram_tensor("wk", [H, GW], F32R, kind="ExternalInput")
    wv = nc.dram_tensor("wv", [H, GW], F32R, kind="ExternalInput")
    wo = nc.dram_tensor("wo", [GW, S], F32R, kind="ExternalInput")
    mask1 = nc.dram_tensor("mask1", [S], F32, kind="ExternalInput")
    bq1 = nc.dram_tensor("bq1", [GW], F32, kind="ExternalInput")
    bk1 = nc.dram_tensor("bk1", [GW], F32, kind="ExternalInput")
    bv1 = nc.dram_tensor("bv1", [1, GW], F32, kind="ExternalInput")
    ones = nc.dram_tensor("ones", [128, TC, HPG, 1], EXPTYPE, kind="ExternalInput")
    onecol = nc.dram_tensor("onecol", [1, HD], F32R, kind="ExternalInput")
    out = nc.dram_tensor("out", [S, H], F32, kind="ExternalOutput")

    with tile.TileContext(nc, pool_alloc_mode="stack") as tc:
      for _rep in range(nrep):
          # Pool releases must be LIFO, so the three big input pools
          # (xT/wqk/wv) are created LAST: they release mid-kernel (stack
          # rewinds) and p_wo then reuses their space.
          misc_cm = tc.tile_pool(name="misc", bufs=1); misc = misc_cm.__enter__()
          qkt_cm = tc.tile_pool(name="p_qkt", bufs=1); p_qkt = qkt_cm.__enter__()
          v_cm = tc.tile_pool(name="p_v", bufs=1); p_v = v_cm.__enter__()
          exp_cm = tc.tile_pool(name="p_exp", bufs=3); p_exp = exp_cm.__enter__()
          attn_cm = tc.tile_pool(name="p_attn", bufs=1)
          p_attn = attn_cm.__enter__()
          nrm_cm = tc.tile_pool(name="p_nrm", bufs=2); p_nrm = nrm_cm.__enter__()
          o_cm = tc.tile_pool(name="p_o", bufs=3); p_o = o_cm.__enter__()
          xT_cm = tc.tile_pool(name="p_xT", bufs=1); p_xT = xT_cm.__enter__()
          wqk_cm = tc.tile_pool(name="p_wqk", bufs=1); p_wqk = wqk_cm.__enter__()
          wv_cm = tc.tile_pool(name="p_wv", bufs=1); p_wv = wv_cm.__enter__()
          late = {"p_nrm": p_nrm}
          qkvps_cm = tc.tile_pool(name="ps_qkv", bufs=2, space="PSUM")
          ps_qkv = qkvps_cm.__enter__()
          lgps_cm = tc.tile_pool(name="ps_lg", bufs=2, space="PSUM")
          ps_lg = lgps_cm.__enter__()
          avps_cm = tc.tile_pool(name="ps_av", bufs=2, space="PSUM")
          ps_av = avps_cm.__enter__()

          # ---- tiny const DMAs first (cheap; they gate exp and copies) ----
          maskb = misc.tile([128, TC], F32, tag="maskb")
          mraw = misc.tile([128, TC], F32, tag="mraw")
          nc.sync.dma_start(out=mraw, in_=mask1.ap().rearrange("(c p) -> p c", p=128))
          nc.vector.tensor_scalar_mul(maskb, mraw, NEG_INF)
          bq_sb = misc.tile([128, 4], F32, tag="bq")
          bk_sb = misc.tile([128, 4], F32, tag="bk")
          nc.sync.dma_start(out=bq_sb, in_=bq1.ap().rearrange("(c p) -> p c", p=128))
          nc.sync.dma_start(out=bk_sb, in_=bk1.ap().rearrange("(c p) -> p c", p=128))
          bv_bc = misc.tile([128, GW], F32, tag="bv")
          nc.sync.dma_start(out=bv_bc, in_=bv1[0:1, :].to_broadcast((128, GW)))
          onecol_sb = misc.tile([1, HD], F32R, tag="onecol")
          nc.sync.dma_start(out=onecol_sb, in_=onecol[:, :])

          # ---- big input loads first: first QT matmul needs xT + wq blk0,
          # so those DMAs get top scheduler priority; weights split per block.
          xT_sb = p_xT.tile([128, HC, S], F32R, tag="xT")
          wq_sb = p_wqk.tile([128, HC, GW], F32R, tag="wq")
          wk_sb = p_wqk.tile([128, HC, GW], F32R, tag="wk")
          wv_sb = p_wv.tile([128, HC, GW], F32R, tag="wv")
          wq_r = wq.ap().rearrange("(c p) m -> p c m", p=128)
          wk_r = wk.ap().rearrange("(c p) m -> p c m", p=128)
          # operands of the very first matmuls first: wq blk0, then xT sh0
          nc.sync.dma_start(out=wq_sb[:, :, 0:128], in_=wq_r[:, :, 0:128])
          for c in range(HC):
              nc.sync.dma_start(out=xT_sb[:, c, 0:512], in_=xT[c * 128:(c + 1) * 128, 0:512])
          nc.sync.dma_start(out=wk_sb[:, :, 0:128], in_=wk_r[:, :, 0:128])
          for c in range(HC):
              nc.sync.dma_start(out=xT_sb[:, c, 512:1024], in_=xT[c * 128:(c + 1) * 128, 512:1024])
          for blk in range(1, 4):
              bs = slice(blk * 128, (blk + 1) * 128)
              nc.sync.dma_start(out=wq_sb[:, :, bs], in_=wq_r[:, :, bs])
          for blk in range(1, 4):
              bs = slice(blk * 128, (blk + 1) * 128)
              nc.sync.dma_start(out=wk_sb[:, :, bs], in_=wk_r[:, :, bs])
          nc.sync.dma_start(out=wv_sb, in_=wv.ap().rearrange("(c p) m -> p c m", p=128))


          QT_sb = p_qkt.tile([128, 4, S], F32R, tag="QT")
          KT_sb = p_qkt.tile([128, 4, S], F32R, tag="KT")
          V_sb = p_v.tile([128, TC, HPG, HD + 1], EXPTYPE, tag="V")
          nc.sync.dma_start(out=V_sb[:, :, :, HD:HD + 1], in_=ones.ap())
          late["attnT"] = p_attn.tile([128, 4, S], F32R, tag="attnT", name="attnT")

          def proj_half(dst, blk, sh, w_sb, b_sb):
              """dst[:, blk, sh-half] (+bias) = block of Wg^T x^T."""
              ps = ps_qkv.tile([128, 512], F32, tag="mm512")
              for c in range(HC):
                  nc.tensor.matmul(
                      ps, w_sb[:, c, blk * 128:(blk + 1) * 128],
                      xT_sb[:, c, sh * 512:(sh + 1) * 512],
                      start=(c == 0), stop=(c == HC - 1))
              nc.vector.tensor_scalar_add(
                  dst[:, blk, sh * 512:(sh + 1) * 512], ps, b_sb[:, blk:blk + 1])

          def v_chunk(tcn):
              """V_sb[:, tcn, :, 0:64] (+bv) = rows 128*tcn.. of x Wv_g."""
              ps = ps_qkv.tile([128, 512], F32, tag="mm512")
              for c in range(HC):
                  nc.tensor.matmul(
                      ps, xT_sb[:, c, tcn * 128:(tcn + 1) * 128], wv_sb[:, c, :],
                      start=(c == 0), stop=(c == HC - 1))
              nc.vector.tensor_add(
                  V_sb[:, tcn, :, 0:HD],
                  ps.rearrange("p (h d) -> p h d", h=HPG),
                  bv_bc.rearrange("p (h d) -> p h d", h=HPG))

          def logits_exp(pair, tcn, exp_dsts, split=False):
              """Packed pair of d=64 logit matmuls + exp for chunk tcn.
              split=True: one exp per s-half, so the first exps don't wait
              for the late-arriving second half of xT."""
              for i, (off, tp) in enumerate(((0, (0, 0)), (64, (64, 0)))):
                  lg = ps_lg.tile([128, 1024], F32, tag="lg")
                  for sh in range(SB):
                      nc.tensor.matmul(
                          lg[:, sh * 512:(sh + 1) * 512],
                          KT_sb[off:off + 64, pair, tcn * 128:(tcn + 1) * 128],
                          QT_sb[off:off + 64, pair, sh * 512:(sh + 1) * 512],
                          start=True, stop=True, tile_position=tp)
                      if split:
                          nc.scalar.activation(
                              out=exp_dsts[i][:, tcn, sh * 512:(sh + 1) * 512],
                              in_=lg[:, sh * 512:(sh + 1) * 512], func=Exp,
                              bias=maskb[:, tcn:tcn + 1], scale=0.125)
                  if not split:
                      nc.scalar.activation(
                          out=exp_dsts[i][:, tcn, :], in_=lg, func=Exp,
                          bias=maskb[:, tcn:tcn + 1], scale=0.125)

          def av_head_half(h, expT_h, sh, tail=False):
              """attnT rows for head h, s-half sh = normalized V_h^T @ expT_h.
              tail=True: route the bcast copy to ScalarE (idle after the last
              exp) so the DVE chain doesn't pace the tail."""
              off = (h % 2) * 64
              if True:
                  pav = ps_av.tile([HD + 1, 512], F32, tag="av",
                                    name=f"pav{h}_{sh}")
                  for tcn in range(TC):
                      nc.tensor.matmul(
                          pav, V_sb[:, tcn, h, :],
                          expT_h[:, tcn, sh * 512:(sh + 1) * 512],
                          start=(tcn == 0), stop=(tcn == TC - 1))
                  recip = late["p_nrm"].tile([1, 512], F32R, tag="recip")
                  with nc.allow_low_precision(reason="softmax denom recip to f32r"):
                      nc.vector.reciprocal(recip, pav[HD:HD + 1, :])
                  bps = ps_qkv.tile([HD, 512], F32, tag="mm512",
                                    name=f"bps{h}_{sh}")
                  nc.tensor.matmul(bps, onecol_sb, recip, start=True, stop=True)
                  bcast = late["p_nrm"].tile([HD, 512], F32, tag="bcast")
                  if tail:
                      nc.scalar.copy(bcast, bps)
                  else:
                      nc.vector.tensor_copy(bcast, bps)
                  nc.vector.tensor_mul(
                      late["attnT"][off:off + HD, h // 2, sh * 512:(sh + 1) * 512],
                      pav[0:HD, :], bcast)

          # ---------------- emission ----------------
          for sh in range(SB):
              proj_half(QT_sb, 0, sh, wq_sb, bq_sb)
          for sh in range(SB):
              proj_half(KT_sb, 0, sh, wk_sb, bk_sb)

          expT = {}
          for pair in range(4):
              ha, hb = 2 * pair, 2 * pair + 1
              if pair >= 1:
                  # free head 2p-2's expT slot ASAP: its exps finished during
                  # the previous loop, and the next pair's second tile waits
                  # on this slot
                  av_head_half(2 * pair - 2, expT[2 * pair - 2], 0)
              expT[ha] = p_exp.tile([128, TC, S], EXPTYPE, tag="expT", name=f"expT{ha}")
              expT[hb] = p_exp.tile([128, TC, S], EXPTYPE, tag="expT", name=f"expT{hb}")
              for tcn in range(TC):
                  # emit independent PE filler BEFORE logits: the logits
                  # matmul waits on its psum slot (paced by ACT exp), and the
                  # PE executes in order, so filler placed after it would
                  # head-of-line block
                  if pair >= 1 and tcn == 1:
                      av_head_half(2 * pair - 2, expT[2 * pair - 2], 1)
                  if pair >= 1 and tcn in (2, 4):
                      av_head_half(2 * pair - 1, expT[2 * pair - 1], tcn // 2 - 1)
                  if pair == 0:
                      if tcn > 0:
                          v_chunk(tcn)
                      if tcn in (1, 3):
                          proj_half(QT_sb, 1, tcn // 2, wq_sb, bq_sb)
                      elif tcn in (5, 7):
                          proj_half(KT_sb, 1, (tcn - 5) // 2, wk_sb, bk_sb)
                  elif pair < 3:
                      if tcn in (1, 3):
                          proj_half(QT_sb, pair + 1, tcn // 2, wq_sb, bq_sb)
                      elif tcn in (5, 7):
                          proj_half(KT_sb, pair + 1, (tcn - 5) // 2, wk_sb, bk_sb)
                  logits_exp(pair, tcn, (expT[ha], expT[hb]))
                  if pair == 0 and tcn == 0:
                      v_chunk(0)

          wv_cm.__exit__(None, None, None)
          wqk_cm.__exit__(None, None, None)
          xT_cm.__exit__(None, None, None)

          wo_cm = tc.tile_pool(name="p_wo", bufs=1)
          p_wo = wo_cm.__enter__()
          wo_sb = p_wo.tile([128, 4, S], F32R, tag="wo")
          nc.sync.dma_start(out=wo_sb, in_=wo.ap().rearrange("(c p) n -> p c n", p=128))

          for h in (6, 7):
              for sh in range(SB):
                  av_head_half(h, expT[h], sh, tail=True)

          attnT = late["attnT"]
          for st in range(TC):
              po = ps_lg.tile([128, 1024], F32, tag="lg", name=f"po{st}")
              for nh in range(SB):
                  for blk in range(4):
                      nc.tensor.matmul(
                          po[:, nh * 512:(nh + 1) * 512],
                          attnT[:, blk, st * 128:(st + 1) * 128],
                          wo_sb[:, blk, nh * 512:(nh + 1) * 512],
                          start=(blk == 0), stop=(blk == 3))
              o_sb = p_o.tile([128, 1024], F32, tag="o")
              nc.scalar.copy(o_sb, po)
              nc.sync.dma_start(out=out[st * 128:(st + 1) * 128, :], in_=o_sb)

          for cm in (wo_cm, o_cm, nrm_cm, attn_cm, exp_cm, v_cm, qkt_cm,
                     misc_cm, avps_cm, lgps_cm, qkvps_cm):
              cm.__exit__(None, None, None)

    nc.compile()
    return nc


_NC = {}


def _get_nc(nrep=1):
    if nrep not in _NC:
        _NC[nrep] = _build(nrep)
    return _NC[nrep]


def kernel(x, mask, Wq, bq, Wk, bk, Wv, bv, Wo, bo, _trace=False):
    x = np.asarray(x, dtype=np.float32)
    mask = np.asarray(mask, dtype=np.float32)
    Wq, Wk, Wv, Wo = (np.asarray(w, dtype=np.float32) for w in (Wq, Wk, Wv, Wo))
    bq, bk, bv, bo = (np.asarray(b_, dtype=np.float32) for b_ in (bq, bk, bv, bo))

    nc = _get_nc()
    ones = np.ones((128, TC, HPG, 1), dtype=ml_dtypes.bfloat16)
    in_maps = []
    for c in range(NCORES):
        b, g = c // 2, c % 2
        sl = slice(g * GW, (g + 1) * GW)
        in_maps.append({
            "xT": np.ascontiguousarray(x[b].T),
            "wq": np.ascontiguousarray(Wq[:, sl]),
            "wk": np.ascontiguousarray(Wk[:, sl]),
            "wv": np.ascontiguousarray(Wv[:, sl]),
            "wo": np.ascontiguousarray(Wo[sl, :]),
            "mask1": np.ascontiguousarray(mask[b, 0, 0, :]),
            "bq1": np.ascontiguousarray(bq[sl]),
            "bk1": np.ascontiguousarray(bk[sl]),
            "bv1": np.ascontiguousarray(bv[sl]).reshape(1, GW),
            "ones": ones,
            "onecol": np.ones((1, HD), np.float32),
        })
    # First execution after NEFF load can race engine table initialization
    # (observed: garbage exp output on run 1 only). Warm up, then run.
    run_bass_kernel_spmd(nc, in_maps, core_ids=list(range(NCORES)))
    res = run_bass_kernel_spmd(
        nc, in_maps, core_ids=list(range(NCORES)), trace=_trace)
    kernel.last_results = res
    parts = [res.results[c]["out"] for c in range(NCORES)]
    return np.stack(
        [parts[2 * b] + parts[2 * b + 1] + bo for b in range(B)]
    ).astype(np.float32)

